# revision 5
# baseline (speedup 1.0000x reference)
"""MultiHuberLoss Trainium2 kernel (v3: sorted-target extraction, tapered tail).

Reference (per element, with m = +x at the target class, -x elsewhere):
    hinge = max(0, 1 - m);  loss = where(m >= -1, hinge^2, -4m);  out = sum(loss)/N

Exact identities used:
  F(-x) = (clamp(x,-1,1) + 1)^2 + 4*relu(x-1)          (main pass, all elements)
  F(x_t) - F(-x_t) = -4 * x_t                          (per-row target correction)
So:  sum(loss) = sum_ij (clamp+1)^2 + 4*sum_ij relu(x-1) - 4*sum_i x[i, target_i]

Data parallel over 8 cores (8192 rows each).  Host-side, each core's rows
are SORTED BY TARGET and laid out so slot (p, j) holds sorted-rank j*128+p.
Then the 128 rows of any j-slot have targets inside a narrow (<64-wide)
column band [c0_j, c0_j+64), so a single 64-wide is_equal mask
(scalar_tensor_tensor) per slot extracts all 128 targets in ~200ns -- no
gpsimd indirect-DMA gathers at all.  Extracted (masked) products are
written into a persistent [128, 64*W] strip and reduced once at the end
(avoids 64 accumulator reads).

Tile sizes taper (7x8000, 4000, 2000, 2000) so the serial v->Square chain
on the last tile adds only ~3us of tail after the final DMA lands.

Per-core engine split:
  - DVE:   v = clamp(x,-1,1) -> bf16 (2x fp32 mode), s = relu(x-1) -> bf16
           on PE-tiles, 64 tiny stt extractions, final reduces
  - ACT:   Square(v + 1) with fused accum -> A;  Relu(x-1)+accum on 2 tiles
  - PE:    B-term sums of s via ones^T @ s matmuls accumulated in PSUM
"""

import numpy as np

import concourse.bacc as bacc
import concourse.bass as bass
import concourse.mybir as mybir
from concourse.bass_utils import run_bass_kernel_spmd
from concourse.tile import TileContext

N_TOTAL = 65536
C = 1000
N_CORES = 8
ROWS = N_TOTAL // N_CORES  # 8192 rows per core
P = 128                    # partitions
JPP = ROWS // P            # 64 rows (slots) per partition
TILE_FD = [8000] * 7 + [4000, 2000, 2000]   # sums to 64000
NT = len(TILE_FD)
W = 64                     # is_equal scan window per slot (band width)
CHUNK = 500                # matmul rhs free-dim chunk (PSUM bank: 500 fp32)

# tiles whose B-term (sum relu(x-1)) runs on ACT; the rest go DVE->PE
ACT_B_TILES = (0, 3)

f32 = mybir.dt.float32
bf16 = mybir.dt.bfloat16
Alu = mybir.AluOpType


def build_program(c0s):
    assert len(c0s) == JPP
    nc = bacc.Bacc(
        "TRN2", target_bir_lowering=False, debug=False, num_devices=N_CORES
    )
    x = nc.dram_tensor("x", [ROWS, C], f32, kind="ExternalInput")
    # target column of each (sorted) row as f32, for the is_equal extraction
    tc_in = nc.dram_tensor("tc", [ROWS], f32, kind="ExternalInput")
    out = nc.dram_tensor("out", [1, 1], f32, kind="ExternalOutput")

    x_flat = x.ap().rearrange("(p j) c -> p (j c)", p=P)  # [128, 64000]
    tc2d = tc_in.ap().rearrange("(p j) -> p j", p=P)      # [128, 64]

    n_mm = sum(fd // CHUNK for t, fd in enumerate(TILE_FD)
               if t not in ACT_B_TILES)

    with TileContext(nc) as tc:
        with (
            tc.tile_pool(name="xp", bufs=3) as xp,
            tc.tile_pool(name="vp", bufs=2) as vp,
            tc.tile_pool(name="sp", bufs=2) as sp,
            tc.tile_pool(name="scr", bufs=1) as scr,
            tc.tile_pool(name="small", bufs=1) as small,
            tc.tile_pool(name="psp", bufs=1, space="PSUM") as psp,
        ):
            ones = small.tile([P, 1], bf16, tag="ones")
            nc.vector.memset(ones[:], 1.0)
            ones_f = small.tile([P, 1], f32, tag="ones_f")
            nc.vector.memset(ones_f[:], 1.0)
            negones = small.tile([P, 1], f32, tag="negones")
            nc.vector.memset(negones[:], -1.0)
            # column-index ramp 0..999, same on every partition (f32 exact)
            ci = small.tile([P, C], f32, tag="ci")
            nc.gpsimd.iota(
                ci[:], pattern=[[1, C]], base=0, channel_multiplier=0,
                allow_small_or_imprecise_dtypes=True,
            )
            tcv = small.tile([P, JPP], f32, tag="tcv")
            nc.sync.dma_start(out=tcv[:], in_=tc2d)

            accA = small.tile([P, NT], f32, tag="accA")
            accB = small.tile([P, len(ACT_B_TILES)], f32, tag="accB")
            # persistent strip of masked target products; one reduce at end
            gs = small.tile([P, JPP * W], f32, tag="gs")
            psB = psp.tile([1, CHUNK], f32, tag="psB")

            mm_i = 0
            bcol = 0
            fd_off = 0   # element offset within the 64000-wide row
            for t, FD in enumerate(TILE_FD):
                RPT = FD // C
                j0 = fd_off // C
                xt = xp.tile([P, FD], f32)
                h = FD // 2
                nc.sync.dma_start(
                    out=xt[:, 0:h], in_=x_flat[:, fd_off:fd_off + h]
                )
                nc.sync.dma_start(
                    out=xt[:, h:FD], in_=x_flat[:, fd_off + h:fd_off + FD]
                )
                v = vp.tile([P, FD], bf16)
                nc.vector.tensor_scalar(
                    v[:], xt[:], -1.0, 1.0, Alu.max, Alu.min
                )
                sq = scr.tile([P, FD], bf16, tag="act_scr")
                nc.scalar.activation(
                    sq[:],
                    v[:],
                    mybir.ActivationFunctionType.Square,
                    bias=1.0,
                    scale=1.0,
                    accum_out=accA[:, t:t + 1],
                )
                if t in ACT_B_TILES:
                    w = scr.tile([P, FD], bf16, tag="act_scr")
                    nc.scalar.activation(
                        w[:],
                        xt[:],
                        mybir.ActivationFunctionType.Relu,
                        bias=negones[:],
                        scale=1.0,
                        accum_out=accB[:, bcol:bcol + 1],
                    )
                    bcol += 1
                else:
                    s = sp.tile([P, FD], bf16)
                    nc.vector.tensor_scalar(
                        s[:], xt[:], 1.0, 0.0, Alu.subtract, Alu.max
                    )
                    for k in range(FD // CHUNK):
                        nc.tensor.matmul(
                            out=psB[:],
                            lhsT=ones[:],
                            rhs=s[:, k * CHUNK:(k + 1) * CHUNK],
                            start=(mm_i == 0),
                            stop=(mm_i == n_mm - 1),
                        )
                        mm_i += 1
                # target extraction: one 64-wide is_equal scan per slot
                for jj in range(RPT):
                    gj = j0 + jj
                    c0 = c0s[gj]
                    nc.vector.scalar_tensor_tensor(
                        out=gs[:, gj * W:(gj + 1) * W],
                        in0=ci[:, c0:c0 + W],
                        scalar=tcv[:, gj:gj + 1],
                        in1=xt[:, jj * C + c0:jj * C + c0 + W],
                        op0=Alu.is_equal,
                        op1=Alu.mult,
                    )
                fd_off += FD

            # ---- final combine ----
            rA = small.tile([P, 1], f32, tag="rA")
            nc.vector.reduce_sum(rA[:], accA[:], axis=mybir.AxisListType.X)
            rB1 = small.tile([P, 1], f32, tag="rB1")
            nc.vector.reduce_sum(rB1[:], accB[:], axis=mybir.AxisListType.X)
            rG = small.tile([P, 1], f32, tag="rG")
            nc.vector.reduce_sum(rG[:], gs[:], axis=mybir.AxisListType.X)
            t1 = small.tile([P, 1], f32, tag="t1")
            nc.vector.tensor_tensor(t1[:], rB1[:], rG[:], Alu.subtract)
            t2 = small.tile([P, 1], f32, tag="t2")
            nc.vector.tensor_scalar(t2[:], t1[:], 4.0, None, Alu.mult)
            u4 = small.tile([P, 1], f32, tag="u4")
            nc.vector.tensor_tensor(u4[:], rA[:], t2[:], Alu.add)
            psS = psp.tile([1, 1], f32, tag="psS")
            nc.tensor.matmul(
                out=psS[:], lhsT=ones_f[:], rhs=u4[:], start=True, stop=True
            )
            # sum of PE-tile B partial sums (PSUM [1, CHUNK])
            rPE = small.tile([1, 1], f32, tag="rPE")
            nc.vector.reduce_sum(rPE[:], psB[:], axis=mybir.AxisListType.X)
            bias_t = small.tile([1, 1], f32, tag="bias_t")
            nc.vector.tensor_scalar(
                bias_t[:], rPE[:], 4.0 / N_TOTAL, None, Alu.mult
            )
            res = small.tile([1, 1], f32, tag="res")
            nc.scalar.activation(
                res[:],
                psS[:],
                mybir.ActivationFunctionType.Identity,
                bias=bias_t[:],
                scale=1.0 / N_TOTAL,
            )
            nc.sync.dma_start(out=out.ap(), in_=res[:])

    nc.compile()
    return nc


_NC_CACHE = {}
LAST_RESULTS = None


def _prep(input, target):
    """Sort each core's rows by target; compute per-slot column bands."""
    x = np.asarray(input, dtype=np.float32)
    tg = np.asarray(target).astype(np.int64)
    xs, ts = [], []
    # slot (p, j) <- sorted rank j*128 + p;  dest row r = p*JPP + j
    r = np.arange(ROWS)
    k_of_r = (r % JPP) * P + r // JPP
    for c in range(N_CORES):
        t_c = tg[c * ROWS:(c + 1) * ROWS]
        order = np.argsort(t_c, kind="stable")
        perm = order[k_of_r]
        xs.append(np.ascontiguousarray(x[c * ROWS:(c + 1) * ROWS][perm]))
        ts.append(t_c[perm])
    tmat = np.stack(ts).reshape(N_CORES, P, JPP)  # [core, p, j]
    tmin = tmat.min(axis=(0, 1))
    tmax = tmat.max(axis=(0, 1))
    c0s = np.clip(tmin, 0, C - W)
    assert (tmax < c0s + W).all(), (
        f"target band wider than W={W}: spans "
        f"{(tmax - c0s + 1).max()}"
    )
    return xs, [t.astype(np.float32) for t in ts], tuple(int(v) for v in c0s)


def kernel(input, target):
    global LAST_RESULTS
    x = np.asarray(input, dtype=np.float32)
    tg = np.asarray(target).astype(np.int64)
    assert x.shape == (N_TOTAL, C), x.shape
    assert tg.shape == (N_TOTAL,), tg.shape

    xs, ts, c0s = _prep(x, tg)
    if c0s not in _NC_CACHE:
        _NC_CACHE[c0s] = build_program(c0s)
    nc = _NC_CACHE[c0s]

    in_maps = [{"x": xs[c], "tc": ts[c]} for c in range(N_CORES)]
    res = run_bass_kernel_spmd(nc, in_maps, core_ids=list(range(N_CORES)))
    LAST_RESULTS = res
    total = np.float32(0.0)
    for r in res.results:
        total += np.float32(r["out"].reshape(()))
    return np.asarray(total, dtype=np.float32)


if __name__ == "__main__":
    rng = np.random.default_rng(0)
    xs = rng.standard_normal((N_TOTAL, C), dtype=np.float32)
    ts = rng.integers(0, C, size=(N_TOTAL,)).astype(np.int64)
    got = kernel(xs, ts)
    m = np.where(np.arange(C)[None, :] == ts[:, None], xs, -xs)
    hinge = np.maximum(0.0, 1.0 - m)
    loss = np.where(m >= -1.0, hinge * hinge, -4.0 * m)
    want = loss.sum(dtype=np.float64) / N_TOTAL
    print("got", got, "want", want, "rel", abs(got - want) / abs(want))
